# revision 2
# baseline (speedup 1.0000x reference)
"""Trainium2 Bass kernel for a 2-layer GCN over 2048 independent 25-node
KNN subgraphs (gnn_message_passing).

Strategy (v4, aggregate-first, LDWEIGHTS-port-lean):
  - Each 25-node subgraph is independent -> the sparse aggregation is a
    dense per-graph 25x25 matmul. Host packs the normalized adjacency
    into block-diagonal 128x128 tiles (5 graphs per tile, rows/cols
    125..127 zero), bf16 everywhere (rel err ~8e-3 << 2e-2 budget).
  - Layer 1 is aggregate-first with a feature-major intermediate:
        zT  = x_tile.T @ at_tile     (stationary = x, moving = at)
        h1  = relu(zT.T @ W0)        (stationary = zT, moving = W0)
    zT's PSUM->SBUF copy moves half the bytes a transform-first q would.
  - Layer-2 aggregation needs only the 5 centers/tile: two tiny matmuls
    (stationary = h1 chunks, moving = atc [128,5]) write into one of two
    block-persistent PSUM banks (block = 28/24 tiles); one copy/block.
  - W1 + Wlin run once per block (2 blocks): block 0 flushes mid-loop
    (pipelined over batches 7-9), block 1 at the end.  This cuts the
    W1/Wlin stationary reloads ~3x vs per-14-tile blocks.
  - The main loop is LDWEIGHTS-port-bound (~95ns per 128-col stationary,
    16 loads per 4-tile batch); everything else (zt cast + relu split
    across vector/scalar, DMA) hides under it.
  - DMA: few big issues (each dma_start costs ~600ns on its engine),
    x + late-at on the sync HW ring, early-at + w0 + atc on the scalar
    HW ring, packed w1|wl on gpsimd's software ring.
  - 4 warm-up matmuls bridge PE activity until real data arrives so the
    HAM clock gate (k=4 -> k=8 after ~2 sustained 3.4us windows) opens
    during the DMA head.
  - Data parallel over 8 cores: 256 graphs (52 tiles) per core.
"""

import os
import sys

import ml_dtypes
import numpy as np

for _p in ("/opt/trn_rl_repo", "/opt/trn_rl_repo/concourse"):
    if _p not in sys.path:
        sys.path.insert(0, _p)

import concourse.bass as bass
import concourse.tile as tile
from concourse import bacc, mybir
from concourse.bass_utils import run_bass_kernel_spmd

NCORES = 8
B = 2048            # graphs
K = 25              # nodes per graph
N = B * K           # 51200
GPC = B // NCORES   # 256 graphs per core
G = 5               # graphs packed per PE tile
P = G * K           # 125 real partitions per tile
PP = 128            # padded partition count
NT = (GPC + G - 1) // G   # 52 tiles per core (last tile: 1 real graph)
CP = 5              # centers per tile
AW = 128            # adjacency tile width (125 block cols + 3 zero pad)
F0 = 128            # input features
F1 = 256            # hidden features
TB = 4              # tiles per batch
NB = NT // TB       # 13 batches
BB = [0, 28, 52]    # W1/Wlin block bounds (block 0 flushes mid-loop)
VSPLIT = 32         # relu cols done on vector engine (rest on scalar)
NWARM = 4           # PE warm-up matmuls (bridge HAM activity until the
                    # first real matmuls; real work opens the clock gate)

_f32 = mybir.dt.float32
_bf16 = mybir.dt.bfloat16

_compiled = {}


def _build_nc():
    nc = bacc.Bacc("TRN2", target_bir_lowering=False, debug=False,
                   num_devices=NCORES)

    x_d = nc.dram_tensor("x", [PP, NT, F0], _bf16, kind="ExternalInput")
    at_d = nc.dram_tensor("at", [PP, NT, AW], _bf16, kind="ExternalInput")
    atc_d = nc.dram_tensor("atc", [PP, NT, CP], _bf16, kind="ExternalInput")
    w0_d = nc.dram_tensor("w0", [F0, F1], _bf16, kind="ExternalInput")
    wpk_d = nc.dram_tensor("wpk", [128, 2 * F1 + 2], _bf16,
                           kind="ExternalInput")
    out_d = nc.dram_tensor("out", [1, NT * CP], _f32, kind="ExternalOutput")

    relu = mybir.ActivationFunctionType.Relu

    with tile.TileContext(nc) as tc:
        with (
            tc.tile_pool(name="const", bufs=1) as cpool,
            tc.tile_pool(name="ztp", bufs=2) as ztp,
            tc.tile_pool(name="h1p", bufs=3) as h1p,
            tc.tile_pool(name="p2p", bufs=2) as p2p,
            tc.tile_pool(name="h3p", bufs=2) as h3p,
            tc.tile_pool(name="outp", bufs=1) as outp,
            tc.tile_pool(name="psum", bufs=1, space=bass.MemorySpace.PSUM) as psp,
        ):
            w0 = cpool.tile([F0, F1], _bf16)
            x_sb = cpool.tile([PP, NT, F0], _bf16)
            at_sb = cpool.tile([PP, NT, AW], _bf16)
            atc_sb = cpool.tile([PP, NT, CP], _bf16)
            wpk = cpool.tile([128, 2 * F1 + 2], _bf16)
            scratch = cpool.tile([128, 512], _bf16)
            out_sb = outp.tile([1, NT * CP], _f32)

            nc.gpsimd.memset(scratch[:], 0.0)

            # ---- DMA issues: each dma_start costs ~600ns on its engine,
            # so use few, large chunks.  sync ring: x + late at chunks.
            # scalar ring: w0 + early at + atc (done before its first
            # relu).  gpsimd (software ring): packed w1|wl, needed late.
            nc.sync.dma_start(x_sb[:, 0:4, :], x_d[:, 0:4, :])
            nc.scalar.dma_start(w0[:], w0_d[:])
            nc.sync.dma_start(x_sb[:, 4:18, :], x_d[:, 4:18, :])
            nc.scalar.dma_start(at_sb[:, 0:4, :], at_d[:, 0:4, :])
            nc.scalar.dma_start(at_sb[:, 4:8, :], at_d[:, 4:8, :])
            nc.scalar.dma_start(atc_sb[:], atc_d[:])
            nc.scalar.dma_start(at_sb[:, 8:18, :], at_d[:, 8:18, :])
            nc.sync.dma_start(at_sb[:, 18:34, :], at_d[:, 18:34, :])
            nc.sync.dma_start(x_sb[:, 18:34, :], x_d[:, 18:34, :])
            nc.sync.dma_start(x_sb[:, 34:52, :], x_d[:, 34:52, :])
            nc.sync.dma_start(at_sb[:, 34:52, :], at_d[:, 34:52, :])
            nc.gpsimd.dma_start(wpk[:], wpk_d[:])

            # ---- PSUM layout (7 of 8 banks):
            #   zt  tag: 2 x [128,4,128] f32 (1 bank each)
            #   h1  tag: 2 x [128,2,256] f32 (1 bank each; half-batches)
            #   p2  tag: 2 x [128,512] f32 (block-persistent + o tail)
            #   fin tag: 1 x [128,512] f32 (h3 per block, reused)
            p2bank = [psp.tile([128, 512], _f32, tag="p2", bufs=2,
                               name=f"p2bank{k}") for k in range(2)]

            # ---- PE warm-up on zeroed scratch (tag h1 bank; real h1
            # allocations reuse it afterwards with tracked deps)
            warm_ps = psp.tile([128, 2, 256], _f32, tag="h1", bufs=2)
            for _ in range(NWARM):
                nc.tensor.matmul(warm_ps[:], scratch[:, 0:128],
                                 scratch[:], start=True, stop=True)

            pending = {}

            def defer(b, fn):
                pending.setdefault(b, []).append(fn)

            def emit_mma(b):
                zt_ps = psp.tile([128, TB, 128], _f32, tag="zt", bufs=2)
                for j in range(TB):
                    i = b * TB + j
                    nc.tensor.matmul(zt_ps[:, j, :], x_sb[:, i, :],
                                     at_sb[:, i, :], start=True, stop=True)
                return zt_ps

            def emit_flush(k, cur):
                # block k's tiles are complete at the end of batch `cur`;
                # pipeline copy -> W1 -> relu|Wlin|out over later batches
                lo, hi = BB[k], BB[k + 1]
                bsz5 = (hi - lo) * CP
                bank = p2bank[k]
                cell = {}

                def do_copy():
                    p2_sb = p2p.tile([128, 2 * bsz5], _bf16, name="p2_sb")
                    nc.vector.tensor_copy(p2_sb[:], bank[:, 0:2 * bsz5])
                    cell["p2_sb"] = p2_sb

                def do_w1():
                    h3_ps = psp.tile([128, 512], _f32, tag="fin", bufs=1,
                                     name="h3_ps")
                    p2_sb = cell["p2_sb"]
                    for foc in range(2):
                        for fic in range(2):
                            nc.tensor.matmul(
                                h3_ps[:, foc * bsz5:(foc + 1) * bsz5],
                                wpk[:, fic * F1 + foc * 128:
                                    fic * F1 + (foc + 1) * 128],
                                p2_sb[:, fic * bsz5:(fic + 1) * bsz5],
                                start=(fic == 0), stop=(fic == 1))
                    cell["h3_ps"] = h3_ps

                def do_out():
                    h3_sb = h3p.tile([128, 2 * bsz5], _bf16, name="h3_sb")
                    nc.scalar.activation(h3_sb[:],
                                         cell["h3_ps"][:, 0:2 * bsz5], relu)
                    o_ps = bank[0:1, 2 * bsz5:3 * bsz5]
                    for foc in range(2):
                        nc.tensor.matmul(o_ps, wpk[:, 2 * F1 + foc:
                                                   2 * F1 + foc + 1],
                                         h3_sb[:, foc * bsz5:
                                               (foc + 1) * bsz5],
                                         start=(foc == 0), stop=(foc == 1))
                    nc.vector.tensor_copy(out_sb[0:1, lo * CP:hi * CP], o_ps)
                    if k == len(BB) - 2:
                        nc.sync.dma_start(out_d[:], out_sb[:])

                defer(cur + 1, do_copy)
                defer(cur + 2, do_w1)
                defer(cur + 3, do_out)

            # ---- software-pipelined main loop ----
            zt_ps = emit_mma(0)
            for b in range(NB):
                zt_sb = ztp.tile([128, TB, 128], _bf16)
                nc.vector.tensor_copy(zt_sb[:], zt_ps[:])

                if b + 1 < NB:
                    zt_ps = emit_mma(b + 1)

                h1_sb = h1p.tile([128, TB, F1], _bf16)
                for j in range(2):   # half-batches of 2 tiles
                    h1_ps = psp.tile([128, 2, F1], _f32, tag="h1", bufs=2)
                    for jj in range(2):
                        nc.tensor.matmul(h1_ps[:, jj, :],
                                         zt_sb[:, 2 * j + jj, :], w0[:],
                                         start=True, stop=True)
                    nc.scalar.activation(h1_sb[:, 2 * j:2 * j + 2, VSPLIT:F1],
                                         h1_ps[:, :, VSPLIT:F1], relu)
                    nc.vector.tensor_scalar_max(
                        h1_sb[:, 2 * j:2 * j + 2, 0:VSPLIT],
                        h1_ps[:, :, 0:VSPLIT], 0.0)
                    for jj in range(2):
                        i = b * TB + 2 * j + jj
                        k = 0 if i < BB[1] else 1
                        off = (i - BB[k]) * CP
                        bsz5 = (BB[k + 1] - BB[k]) * CP
                        for fic in range(2):
                            nc.tensor.matmul(
                                p2bank[k][:, fic * bsz5 + off:
                                          fic * bsz5 + off + CP],
                                h1_sb[:, 2 * j + jj,
                                      fic * 128:(fic + 1) * 128],
                                atc_sb[:, i, :],
                                start=True, stop=True)
                        if i + 1 == BB[k + 1]:
                            emit_flush(k, b)

                for fn in pending.pop(b, []):
                    fn()

            for b in sorted(list(pending)):
                for fn in pending.pop(b):
                    fn()

    nc.compile()
    return nc


def _get_nc(mode=None):
    if "v4" not in _compiled:
        _compiled["v4"] = _build_nc()
    return _compiled["v4"]


def _host_prep(x, edge_weight, W0, W1, Wlin, edge_index):
    bf = ml_dtypes.bfloat16
    src = edge_index[0].astype(np.int64)
    tgt = edge_index[1].astype(np.int64)
    b = src // K
    sl = src - b * K
    tl = tgt - (tgt // K) * K

    # dense raw adjacency per graph, indexed [b, t, s]
    idx = (b * K + tl) * K + sl
    Araw = np.bincount(idx, weights=edge_weight.astype(np.float64),
                       minlength=B * K * K).astype(np.float32).reshape(B, K, K)
    deg = Araw.sum(axis=2)                      # weighted in-degree [B, K]
    with np.errstate(divide="ignore"):
        dinv = np.where(deg > 0, 1.0 / np.sqrt(deg), 0.0).astype(np.float32)
    An = Araw * dinv[:, :, None] * dinv[:, None, :]   # [b, t, s]
    ATn = np.ascontiguousarray(An.transpose(0, 2, 1))  # [b, s, t]

    # scatter graphs into per-core padded tile slots
    SLOTS = NT * G
    ATs = np.zeros((NCORES, SLOTS, K, K), np.float32)
    ATs[:, :GPC] = ATn.reshape(NCORES, GPC, K, K)
    ATs = ATs.reshape(NCORES, NT, G, K, K)

    at = np.zeros((NCORES, NT, PP, AW), np.float32)
    bd = at[:, :, :P, :P].reshape(NCORES, NT, G, K, G, K)
    atc = np.zeros((NCORES, NT, PP, CP), np.float32)
    cent = atc[:, :, :P, :G].reshape(NCORES, NT, G, K, G)
    for g in range(G):
        bd[:, :, g, :, g, :] = ATs[:, :, g]          # block-diagonal AT
        cent[:, :, g, :, g] = ATs[:, :, g, :, 0]     # center (t_local=0) col
    # device layout [PP, NT, .]
    at = np.ascontiguousarray(at.transpose(0, 2, 1, 3).astype(bf))
    atc = np.ascontiguousarray(atc.transpose(0, 2, 1, 3).astype(bf))

    # node-major x, tiled and padded: x_nm[p, i, f] = x[i*P + p, f], p < 125
    xp = np.zeros((NCORES, NT, PP, F0), np.float32)
    xtmp = np.zeros((NCORES, NT * P, F0), np.float32)
    xtmp[:, :GPC * K] = x.reshape(NCORES, GPC * K, F0)
    xp[:, :, :P, :] = xtmp.reshape(NCORES, NT, P, F0)
    x_nm = np.ascontiguousarray(xp.transpose(0, 2, 1, 3).astype(bf))

    # packed [w1 fic0 | w1 fic1 | wl]: wpk[p, fic*256+fo] = W1[fic*128+p, fo]
    wpk = np.empty((128, 2 * F1 + 2), np.float32)
    w1p = W1.reshape(2, 128, F1).transpose(1, 0, 2)   # [128, fic, fo]
    wpk[:, 0:F1] = w1p[:, 0, :]
    wpk[:, F1:2 * F1] = w1p[:, 1, :]
    wpk[:, 2 * F1:] = Wlin.reshape(2, 128).T          # [128, foc]
    wpk = np.ascontiguousarray(wpk.astype(bf))

    in_maps = []
    for c in range(NCORES):
        in_maps.append({
            "x": x_nm[c],
            "at": np.ascontiguousarray(at[c]),
            "atc": np.ascontiguousarray(atc[c]),
            "w0": np.ascontiguousarray(W0.astype(bf)),
            "wpk": wpk,
        })
    return in_maps


def _run(inputs, mode=None, trace=False):
    nc = _get_nc()
    in_maps = _host_prep(**inputs)
    res = run_bass_kernel_spmd(nc, in_maps, core_ids=list(range(NCORES)),
                               trace=trace)
    out = np.empty((B, 1), np.float32)
    for c in range(NCORES):
        vals = res.results[c]["out"].reshape(-1)
        out[c * GPC:(c + 1) * GPC, 0] = vals[:GPC]
    return out, res


def kernel(**inputs):
    out, _ = _run(inputs, trace=False)
    return out


# revision 3
# speedup vs baseline: 1.3678x; 1.3678x over previous
"""Trainium2 Bass kernel for a 2-layer GCN over 2048 independent 25-node
KNN subgraphs (gnn_message_passing).

Strategy (v5, aggregate-first, LDWEIGHTS-port-lean, stall-free pipeline):
  - Each 25-node subgraph is independent -> the sparse aggregation is a
    dense per-graph 25x25 matmul. Host packs the normalized adjacency
    into block-diagonal 128x128 tiles (5 graphs per tile), bf16.
  - Layer 1 aggregate-first:  zT = x.T @ at;  h1 = relu(zT.T @ W0).
  - Layer-2 center aggregation: p2 = h1.T @ atc (2 matmuls/tile, 5 mov
    cols) into block-persistent PSUM banks; W1+Wlin once per block
    (2 blocks), reusing the block's own bank for h3/o after its copy.
  - Pipeline (per iteration b): mmA(b+1) -> cast zt(b+1) [vector] ->
    h1(b) -> relu(b) [scalar 224 cols + vector 32] -> p2(b-1).
    p2 is deferred one batch so it never waits on relu; the cast runs
    one batch ahead so h1 never waits on vector.  Steady state is
    LDWEIGHTS-port-bound (~16 x 95ns per 4-tile batch).
  - PSUM: zt 2 banks (bufs=2), h1 2x2 banks (bufs=2), p2 2 banks. = 8.
  - DMA: few big issues (a dma_start costs ~600-750ns on its engine);
    x + late-at on the sync HW ring, w0/atc/mid-at on the scalar HW
    ring (both stream in parallel), packed w1|wl on gpsimd.
  - 4 warm-up matmuls bridge PE activity from the init barrier until
    real data arrives, so the HAM k=4->k=8 clock gate opens during the
    DMA head.
  - Data parallel over 8 cores: 256 graphs (52 tiles) per core.
"""

import os
import sys

import ml_dtypes
import numpy as np

for _p in ("/opt/trn_rl_repo", "/opt/trn_rl_repo/concourse"):
    if _p not in sys.path:
        sys.path.insert(0, _p)

import concourse.bass as bass
import concourse.tile as tile
from concourse import bacc, mybir
from concourse.bass_utils import run_bass_kernel_spmd

NCORES = 8
B = 2048            # graphs
K = 25              # nodes per graph
N = B * K           # 51200
GPC = B // NCORES   # 256 graphs per core
G = 5               # graphs packed per PE tile
P = G * K           # 125 real partitions per tile
PP = 128            # padded partition count
NT = (GPC + G - 1) // G   # 52 tiles per core (last tile: 1 real graph)
CP = 5              # centers per tile
AW = 128            # adjacency tile width (125 block cols + 3 zero pad)
F0 = 128            # input features
F1 = 256            # hidden features
TB = 4              # tiles per batch
NB = NT // TB       # 13 batches
BB = [0, 28, 52]    # W1/Wlin block bounds (block 0 flushes mid-loop)
VSPLIT = 32         # relu cols done on vector engine (rest on scalar)
NWARM = 4           # PE warm-up matmuls (bridge HAM activity until the
                    # first real matmuls keep the PE dense)

_f32 = mybir.dt.float32
_bf16 = mybir.dt.bfloat16

_compiled = {}


def _build_nc():
    nc = bacc.Bacc("TRN2", target_bir_lowering=False, debug=False,
                   num_devices=NCORES)

    x_d = nc.dram_tensor("x", [PP, NT, F0], _bf16, kind="ExternalInput")
    at_d = nc.dram_tensor("at", [PP, NT, AW], _bf16, kind="ExternalInput")
    atc_d = nc.dram_tensor("atc", [PP, NT, CP], _bf16, kind="ExternalInput")
    w0_d = nc.dram_tensor("w0", [F0, F1], _bf16, kind="ExternalInput")
    wpk_d = nc.dram_tensor("wpk", [128, 2 * F1 + 2], _bf16,
                           kind="ExternalInput")
    out_d = nc.dram_tensor("out", [1, NT * CP], _f32, kind="ExternalOutput")

    relu = mybir.ActivationFunctionType.Relu

    with tile.TileContext(nc) as tc:
        with (
            tc.tile_pool(name="const", bufs=1) as cpool,
            tc.tile_pool(name="ztp", bufs=3) as ztp,
            tc.tile_pool(name="h1p", bufs=3) as h1p,
            tc.tile_pool(name="p2p", bufs=2) as p2p,
            tc.tile_pool(name="h3p", bufs=2) as h3p,
            tc.tile_pool(name="outp", bufs=1) as outp,
            tc.tile_pool(name="psum", bufs=1, space=bass.MemorySpace.PSUM) as psp,
        ):
            w0 = cpool.tile([F0, F1], _bf16)
            x_sb = cpool.tile([PP, NT, F0], _bf16)
            at_sb = cpool.tile([PP, NT, AW], _bf16)
            atc_sb = cpool.tile([PP, NT, CP], _bf16)
            wpk = cpool.tile([128, 2 * F1 + 2], _bf16)
            scratch = cpool.tile([128, 512], _bf16)
            out_sb = outp.tile([1, NT * CP], _f32)

            nc.gpsimd.memset(scratch[:], 0.0)

            # ---- DMA issues (~600-750ns each on the issuing engine).
            # sync HW ring: x + tail at; scalar HW ring: w0, atc, mid at
            # (streams in parallel with sync's ring, and is done before
            # scalar's first relu); gpsimd software ring: packed w1|wl.
            nc.sync.dma_start(x_sb[:, 0:4, :], x_d[:, 0:4, :])
            nc.scalar.dma_start(w0[:], w0_d[:])
            nc.sync.dma_start(at_sb[:, 0:4, :], at_d[:, 0:4, :])
            nc.scalar.dma_start(atc_sb[:], atc_d[:])
            nc.sync.dma_start(x_sb[:, 4:18, :], x_d[:, 4:18, :])
            nc.scalar.dma_start(at_sb[:, 4:18, :], at_d[:, 4:18, :])
            nc.sync.dma_start(x_sb[:, 18:34, :], x_d[:, 18:34, :])
            nc.scalar.dma_start(at_sb[:, 18:34, :], at_d[:, 18:34, :])
            nc.sync.dma_start(x_sb[:, 34:52, :], x_d[:, 34:52, :])
            nc.sync.dma_start(at_sb[:, 34:52, :], at_d[:, 34:52, :])
            nc.gpsimd.dma_start(wpk[:], wpk_d[:])

            # ---- PSUM layout (8 banks):
            #   zt  tag: 2 x [128,4,128] f32 (1 bank each)
            #   h1  tag: 2 x [128,4,256] f32 (2 banks each; warm shares)
            #   p2  tag: 2 x [128,512] f32 (block p2 + h3/o reuse)
            p2bank = [psp.tile([128, 512], _f32, tag="p2", bufs=2,
                               name=f"p2bank{k}") for k in range(2)]

            warm_ps = psp.tile([128, TB, F1], _f32, tag="h1", bufs=2)
            for _ in range(NWARM):
                nc.tensor.matmul(warm_ps[:, 0, :], scratch[:, 0:128],
                                 scratch[:, 0:256], start=True, stop=True)

            pending = {}

            def defer(b, fn):
                pending.setdefault(b, []).append(fn)

            def emit_mma(b):
                zt_ps = psp.tile([128, TB, 128], _f32, tag="zt", bufs=2)
                for j in range(TB):
                    i = b * TB + j
                    nc.tensor.matmul(zt_ps[:, j, :], x_sb[:, i, :],
                                     at_sb[:, i, :], start=True, stop=True)
                return zt_ps

            def emit_p2(b):
                # layer-2 center aggregation for batch b (deferred one
                # batch so it never waits on relu)
                for jj in range(TB):
                    i = b * TB + jj
                    k = 0 if i < BB[1] else 1
                    bsz5 = (BB[k + 1] - BB[k]) * CP
                    off = (i - BB[k]) * CP
                    for fic in range(2):
                        nc.tensor.matmul(
                            p2bank[k][:, fic * bsz5 + off:
                                      fic * bsz5 + off + CP],
                            h1_sbs[b % 3][:, jj, fic * 128:(fic + 1) * 128],
                            atc_sb[:, i, :],
                            start=True, stop=True)
                    if i + 1 == BB[k + 1]:
                        emit_flush(k)

            def emit_flush(k):
                # block k's p2 bank is complete; pipeline copy -> W1 ->
                # relu|Wlin|out over the following iterations (or the
                # tail for the last block)
                lo, hi = BB[k], BB[k + 1]
                bsz5 = (hi - lo) * CP
                bank = p2bank[k]
                cell = {}

                def do_copy():
                    p2_sb = p2p.tile([128, 2 * bsz5], _bf16, name="p2_sb")
                    nc.vector.tensor_copy(p2_sb[:], bank[:, 0:2 * bsz5])
                    cell["p2_sb"] = p2_sb

                def do_w1():
                    # h3 reuses block k's own bank (p2 data dead after
                    # the copy); o lives past the h3 columns
                    p2_sb = cell["p2_sb"]
                    for foc in range(2):
                        for fic in range(2):
                            nc.tensor.matmul(
                                bank[:, foc * bsz5:(foc + 1) * bsz5],
                                wpk[:, fic * F1 + foc * 128:
                                    fic * F1 + (foc + 1) * 128],
                                p2_sb[:, fic * bsz5:(fic + 1) * bsz5],
                                start=(fic == 0), stop=(fic == 1))

                def do_out():
                    h3_sb = h3p.tile([128, 2 * bsz5], _bf16, name="h3_sb")
                    nc.scalar.activation(h3_sb[:], bank[:, 0:2 * bsz5], relu)
                    o_ps = bank[0:1, 2 * bsz5:3 * bsz5]
                    for foc in range(2):
                        nc.tensor.matmul(o_ps, wpk[:, 2 * F1 + foc:
                                                   2 * F1 + foc + 1],
                                         h3_sb[:, foc * bsz5:
                                               (foc + 1) * bsz5],
                                         start=(foc == 0), stop=(foc == 1))
                    nc.vector.tensor_copy(out_sb[0:1, lo * CP:hi * CP], o_ps)
                    if k == len(BB) - 2:
                        nc.sync.dma_start(out_d[:], out_sb[:])

                base = cur_b[0]
                defer(base + 1, do_copy)
                defer(base + 2, do_w1)
                defer(base + 3, do_out)

            # ---- software-pipelined main loop ----
            h1_sbs = {}
            cur_b = [0]
            zt_sbs = {}
            zt_ps = emit_mma(0)
            zt_sbs[0] = ztp.tile([128, TB, 128], _bf16, name="zt_sb")
            nc.vector.tensor_copy(zt_sbs[0][:], zt_ps[:])

            for b in range(NB):
                cur_b[0] = b
                if b + 1 < NB:
                    zt_ps = emit_mma(b + 1)
                    zt_sbs[(b + 1) % 3] = ztp.tile([128, TB, 128], _bf16,
                                                   name="zt_sb")
                    nc.vector.tensor_copy(zt_sbs[(b + 1) % 3][:], zt_ps[:])

                zt_sb = zt_sbs[b % 3]
                h1_ps = psp.tile([128, TB, F1], _f32, tag="h1", bufs=2)
                for j in range(TB):
                    nc.tensor.matmul(h1_ps[:, j, :], zt_sb[:, j, :], w0[:],
                                     start=True, stop=True)
                h1_sb = h1p.tile([128, TB, F1], _bf16)
                h1_sbs[b % 3] = h1_sb
                nc.scalar.activation(h1_sb[:, :, VSPLIT:F1],
                                     h1_ps[:, :, VSPLIT:F1], relu)
                nc.vector.tensor_scalar_max(h1_sb[:, :, 0:VSPLIT],
                                            h1_ps[:, :, 0:VSPLIT], 0.0)
                if b > 0:
                    emit_p2(b - 1)
                for fn in pending.pop(b, []):
                    fn()

            cur_b[0] = NB
            emit_p2(NB - 1)
            for b in sorted(list(pending)):
                for fn in pending.pop(b):
                    fn()

    nc.compile()
    return nc


def _get_nc(mode=None):
    if "v5" not in _compiled:
        _compiled["v5"] = _build_nc()
    return _compiled["v5"]


def _host_prep(x, edge_weight, W0, W1, Wlin, edge_index):
    bf = ml_dtypes.bfloat16
    src = edge_index[0].astype(np.int64)
    tgt = edge_index[1].astype(np.int64)
    b = src // K
    sl = src - b * K
    tl = tgt - (tgt // K) * K

    # dense raw adjacency per graph, indexed [b, t, s]
    idx = (b * K + tl) * K + sl
    Araw = np.bincount(idx, weights=edge_weight.astype(np.float64),
                       minlength=B * K * K).astype(np.float32).reshape(B, K, K)
    deg = Araw.sum(axis=2)                      # weighted in-degree [B, K]
    with np.errstate(divide="ignore"):
        dinv = np.where(deg > 0, 1.0 / np.sqrt(deg), 0.0).astype(np.float32)
    An = Araw * dinv[:, :, None] * dinv[:, None, :]   # [b, t, s]
    ATn = np.ascontiguousarray(An.transpose(0, 2, 1))  # [b, s, t]

    # scatter graphs into per-core padded tile slots
    SLOTS = NT * G
    ATs = np.zeros((NCORES, SLOTS, K, K), np.float32)
    ATs[:, :GPC] = ATn.reshape(NCORES, GPC, K, K)
    ATs = ATs.reshape(NCORES, NT, G, K, K)

    at = np.zeros((NCORES, NT, PP, AW), np.float32)
    bd = at[:, :, :P, :P].reshape(NCORES, NT, G, K, G, K)
    atc = np.zeros((NCORES, NT, PP, CP), np.float32)
    cent = atc[:, :, :P, :G].reshape(NCORES, NT, G, K, G)
    for g in range(G):
        bd[:, :, g, :, g, :] = ATs[:, :, g]          # block-diagonal AT
        cent[:, :, g, :, g] = ATs[:, :, g, :, 0]     # center (t_local=0) col
    # device layout [PP, NT, .]
    at = np.ascontiguousarray(at.transpose(0, 2, 1, 3).astype(bf))
    atc = np.ascontiguousarray(atc.transpose(0, 2, 1, 3).astype(bf))

    # node-major x, tiled and padded: x_nm[p, i, f] = x[i*P + p, f], p < 125
    xp = np.zeros((NCORES, NT, PP, F0), np.float32)
    xtmp = np.zeros((NCORES, NT * P, F0), np.float32)
    xtmp[:, :GPC * K] = x.reshape(NCORES, GPC * K, F0)
    xp[:, :, :P, :] = xtmp.reshape(NCORES, NT, P, F0)
    x_nm = np.ascontiguousarray(xp.transpose(0, 2, 1, 3).astype(bf))

    # packed [w1 fic0 | w1 fic1 | wl]: wpk[p, fic*256+fo] = W1[fic*128+p, fo]
    wpk = np.empty((128, 2 * F1 + 2), np.float32)
    w1p = W1.reshape(2, 128, F1).transpose(1, 0, 2)   # [128, fic, fo]
    wpk[:, 0:F1] = w1p[:, 0, :]
    wpk[:, F1:2 * F1] = w1p[:, 1, :]
    wpk[:, 2 * F1:] = Wlin.reshape(2, 128).T          # [128, foc]
    wpk = np.ascontiguousarray(wpk.astype(bf))

    in_maps = []
    for c in range(NCORES):
        in_maps.append({
            "x": x_nm[c],
            "at": np.ascontiguousarray(at[c]),
            "atc": np.ascontiguousarray(atc[c]),
            "w0": np.ascontiguousarray(W0.astype(bf)),
            "wpk": wpk,
        })
    return in_maps


def _run(inputs, mode=None, trace=False):
    nc = _get_nc()
    in_maps = _host_prep(**inputs)
    res = run_bass_kernel_spmd(nc, in_maps, core_ids=list(range(NCORES)),
                               trace=trace)
    out = np.empty((B, 1), np.float32)
    for c in range(NCORES):
        vals = res.results[c]["out"].reshape(-1)
        out[c * GPC:(c + 1) * GPC, 0] = vals[:GPC]
    return out, res


def kernel(**inputs):
    out, _ = _run(inputs, trace=False)
    return out


# revision 5
# speedup vs baseline: 1.4999x; 1.0966x over previous
"""Trainium2 Bass kernel for a 2-layer GCN over 2048 independent 25-node
KNN subgraphs (gnn_message_passing).

Strategy (v5, aggregate-first, LDWEIGHTS-port-lean, stall-free pipeline):
  - Each 25-node subgraph is independent -> the sparse aggregation is a
    dense per-graph 25x25 matmul. Host packs the normalized adjacency
    into block-diagonal 128x128 tiles (5 graphs per tile), bf16.
  - Layer 1 aggregate-first:  zT = x.T @ at;  h1 = relu(zT.T @ W0).
  - Layer-2 center aggregation: p2 = h1.T @ atc (2 matmuls/tile, 5 mov
    cols) into block-persistent PSUM banks; W1+Wlin once per block
    (2 blocks), reusing the block's own bank for h3/o after its copy.
  - Pipeline (per iteration b): mmA(b+1) -> cast zt(b+1) [vector] ->
    h1(b) -> relu(b) [scalar 224 cols + vector 32] -> p2(b-1).
    p2 is deferred one batch so it never waits on relu; the cast runs
    one batch ahead so h1 never waits on vector.  Steady state is
    LDWEIGHTS-port-bound (~16 x 95ns per 4-tile batch).
  - PSUM: zt 2 banks (bufs=2), h1 2x2 banks (bufs=2), p2 2 banks. = 8.
  - DMA: few big issues (a dma_start costs ~600-750ns on its engine);
    x + late-at on the sync HW ring, w0/atc/mid-at on the scalar HW
    ring (both stream in parallel), packed w1|wl on gpsimd.
  - 4 warm-up matmuls bridge PE activity from the init barrier until
    real data arrives, so the HAM k=4->k=8 clock gate opens during the
    DMA head.
  - Data parallel over 8 cores: 256 graphs (52 tiles) per core.
"""

import os
import sys

import ml_dtypes
import numpy as np

for _p in ("/opt/trn_rl_repo", "/opt/trn_rl_repo/concourse"):
    if _p not in sys.path:
        sys.path.insert(0, _p)

import concourse.bass as bass
import concourse.tile as tile
from concourse import bacc, mybir
from concourse.bass_utils import run_bass_kernel_spmd

NCORES = 8
B = 2048            # graphs
K = 25              # nodes per graph
N = B * K           # 51200
GPC = B // NCORES   # 256 graphs per core
G = 5               # graphs packed per PE tile
P = G * K           # 125 real partitions per tile
PP = 128            # padded partition count
NT = (GPC + G - 1) // G   # 52 tiles per core (last tile: 1 real graph)
CP = 5              # centers per tile
AW = 128            # adjacency tile width (125 block cols + 3 zero pad)
F0 = 128            # input features
F1 = 256            # hidden features
TB = 4              # tiles per batch
NB = NT // TB       # 13 batches
BB = [0, 34, 52]    # W1/Wlin block bounds (block 0 flushes mid-loop)
VSPLIT = 32         # relu cols done on vector engine (rest on scalar)
NWARM = 16          # PE warm-up matmuls: a dense ~7us (k=4) stream that
                    # opens the HAM clock gate during the DMA head, so
                    # real matmuls run at k=8 (measured: gate opens after
                    # ~4.6us of sustained activity; a sparse ramp never
                    # opens it and the whole loop runs at half clock)

_f32 = mybir.dt.float32
_bf16 = mybir.dt.bfloat16

_compiled = {}


def _build_nc():
    nc = bacc.Bacc("TRN2", target_bir_lowering=False, debug=False,
                   num_devices=NCORES)

    x_d = nc.dram_tensor("x", [PP, NT, F0], _bf16, kind="ExternalInput")
    at_d = nc.dram_tensor("at", [PP, NT, AW], _bf16, kind="ExternalInput")
    atc_d = nc.dram_tensor("atc", [PP, NT, CP], _bf16, kind="ExternalInput")
    w0_d = nc.dram_tensor("w0", [F0, F1], _bf16, kind="ExternalInput")
    wpk_d = nc.dram_tensor("wpk", [128, 2 * F1 + 2], _bf16,
                           kind="ExternalInput")
    out_d = nc.dram_tensor("out", [1, NT * CP], _f32, kind="ExternalOutput")

    relu = mybir.ActivationFunctionType.Relu

    with tile.TileContext(nc) as tc:
        with (
            tc.tile_pool(name="const", bufs=1) as cpool,
            tc.tile_pool(name="ztp", bufs=3) as ztp,
            tc.tile_pool(name="h1p", bufs=3) as h1p,
            tc.tile_pool(name="p2p", bufs=2) as p2p,
            tc.tile_pool(name="h3p", bufs=2) as h3p,
            tc.tile_pool(name="outp", bufs=1) as outp,
            tc.tile_pool(name="psum", bufs=1, space=bass.MemorySpace.PSUM) as psp,
        ):
            w0 = cpool.tile([F0, F1], _bf16)
            x_sb = cpool.tile([PP, NT, F0], _bf16)
            at_sb = cpool.tile([PP, NT, AW], _bf16)
            atc_sb = cpool.tile([PP, NT, CP], _bf16)
            wpk = cpool.tile([128, 2 * F1 + 2], _bf16)
            scratch = cpool.tile([128, 512], _bf16)
            out_sb = outp.tile([1, NT * CP], _f32)

            nc.gpsimd.memset(scratch[:], 0.0)

            # ---- DMA issues (~600-750ns each on the issuing engine).
            # sync HW ring: x + tail at; scalar HW ring: w0, atc, mid at
            # (streams in parallel with sync's ring, and is done before
            # scalar's first relu); gpsimd software ring: packed w1|wl.
            nc.sync.dma_start(x_sb[:, 0:4, :], x_d[:, 0:4, :])
            nc.scalar.dma_start(w0[:], w0_d[:])
            nc.sync.dma_start(at_sb[:, 0:4, :], at_d[:, 0:4, :])
            nc.scalar.dma_start(atc_sb[:], atc_d[:])
            nc.sync.dma_start(x_sb[:, 4:18, :], x_d[:, 4:18, :])
            nc.scalar.dma_start(at_sb[:, 4:18, :], at_d[:, 4:18, :])
            nc.sync.dma_start(x_sb[:, 18:34, :], x_d[:, 18:34, :])
            nc.scalar.dma_start(at_sb[:, 18:34, :], at_d[:, 18:34, :])
            nc.sync.dma_start(x_sb[:, 34:52, :], x_d[:, 34:52, :])
            nc.sync.dma_start(at_sb[:, 34:52, :], at_d[:, 34:52, :])
            nc.gpsimd.dma_start(wpk[:], wpk_d[:])

            # ---- PSUM layout (8 banks):
            #   zt  tag: 2 x [128,4,128] f32 (1 bank each)
            #   h1  tag: 2 x [128,4,256] f32 (2 banks each; warm shares)
            #   p2  tag: 2 x [128,512] f32 (block p2 + h3/o reuse)
            p2bank = [psp.tile([128, 512], _f32, tag="p2", bufs=2,
                               name=f"p2bank{k}") for k in range(2)]

            warm_ps = psp.tile([128, TB, F1], _f32, tag="h1", bufs=2)
            for _ in range(NWARM):
                nc.tensor.matmul(warm_ps[:, 0:2, :], scratch[:, 0:128],
                                 scratch[:], start=True, stop=True)

            pending = {}

            def defer(b, fn):
                pending.setdefault(b, []).append(fn)

            def emit_mma(b):
                zt_ps = psp.tile([128, TB, 128], _f32, tag="zt", bufs=2)
                for j in range(TB):
                    i = b * TB + j
                    nc.tensor.matmul(zt_ps[:, j, :], x_sb[:, i, :],
                                     at_sb[:, i, :], start=True, stop=True)
                return zt_ps

            def emit_p2(b):
                # layer-2 center aggregation for batch b (deferred one
                # batch so it never waits on relu)
                for jj in range(TB):
                    i = b * TB + jj
                    k = 0 if i < BB[1] else 1
                    bsz5 = (BB[k + 1] - BB[k]) * CP
                    off = (i - BB[k]) * CP
                    for fic in range(2):
                        nc.tensor.matmul(
                            p2bank[k][:, fic * bsz5 + off:
                                      fic * bsz5 + off + CP],
                            h1_sbs[b % 3][:, jj, fic * 128:(fic + 1) * 128],
                            atc_sb[:, i, :],
                            start=True, stop=True)
                    if i + 1 == BB[k + 1]:
                        emit_flush(k)

            def emit_flush(k):
                # block k's p2 bank is complete; pipeline copy -> W1 ->
                # relu|Wlin|out over the following iterations (or the
                # tail for the last block)
                lo, hi = BB[k], BB[k + 1]
                bsz5 = (hi - lo) * CP
                bank = p2bank[k]
                cell = {}

                def do_copy():
                    p2_sb = p2p.tile([128, 2 * bsz5], _bf16, name="p2_sb")
                    nc.vector.tensor_copy(p2_sb[:], bank[:, 0:2 * bsz5])
                    cell["p2_sb"] = p2_sb

                def do_w1():
                    # h3 reuses block k's own bank (p2 data dead after
                    # the copy); o lives past the h3 columns
                    p2_sb = cell["p2_sb"]
                    for foc in range(2):
                        for fic in range(2):
                            nc.tensor.matmul(
                                bank[:, foc * bsz5:(foc + 1) * bsz5],
                                wpk[:, fic * F1 + foc * 128:
                                    fic * F1 + (foc + 1) * 128],
                                p2_sb[:, fic * bsz5:(fic + 1) * bsz5],
                                start=(fic == 0), stop=(fic == 1))

                def do_out():
                    h3_sb = h3p.tile([128, 2 * bsz5], _bf16, name="h3_sb")
                    nc.scalar.activation(h3_sb[:], bank[:, 0:2 * bsz5], relu)
                    o_ps = bank[0:1, 2 * bsz5:3 * bsz5]
                    for foc in range(2):
                        nc.tensor.matmul(o_ps, wpk[:, 2 * F1 + foc:
                                                   2 * F1 + foc + 1],
                                         h3_sb[:, foc * bsz5:
                                               (foc + 1) * bsz5],
                                         start=(foc == 0), stop=(foc == 1))
                    nc.vector.tensor_copy(out_sb[0:1, lo * CP:hi * CP], o_ps)
                    if k == len(BB) - 2:
                        nc.sync.dma_start(out_d[:], out_sb[:])

                base = cur_b[0]
                defer(base + 1, do_copy)
                defer(base + 2, do_w1)
                defer(base + 3, do_out)

            # ---- software-pipelined main loop ----
            h1_sbs = {}
            cur_b = [0]
            zt_sbs = {}
            zt_ps = emit_mma(0)
            zt_sbs[0] = ztp.tile([128, TB, 128], _bf16, name="zt_sb")
            nc.vector.tensor_copy(zt_sbs[0][:], zt_ps[:])

            for b in range(NB):
                cur_b[0] = b
                if b + 1 < NB:
                    zt_ps = emit_mma(b + 1)
                    zt_sbs[(b + 1) % 3] = ztp.tile([128, TB, 128], _bf16,
                                                   name="zt_sb")
                    nc.vector.tensor_copy(zt_sbs[(b + 1) % 3][:], zt_ps[:])

                zt_sb = zt_sbs[b % 3]
                h1_ps = psp.tile([128, TB, F1], _f32, tag="h1", bufs=2)
                for j in range(TB):
                    nc.tensor.matmul(h1_ps[:, j, :], zt_sb[:, j, :], w0[:],
                                     start=True, stop=True)
                h1_sb = h1p.tile([128, TB, F1], _bf16)
                h1_sbs[b % 3] = h1_sb
                nc.scalar.activation(h1_sb[:, :, VSPLIT:F1],
                                     h1_ps[:, :, VSPLIT:F1], relu)
                nc.vector.tensor_scalar_max(h1_sb[:, :, 0:VSPLIT],
                                            h1_ps[:, :, 0:VSPLIT], 0.0)
                if b > 0:
                    emit_p2(b - 1)
                for fn in pending.pop(b, []):
                    fn()

            cur_b[0] = NB
            emit_p2(NB - 1)
            for b in sorted(list(pending)):
                for fn in pending.pop(b):
                    fn()

    nc.compile()
    return nc


def _get_nc(mode=None):
    if "v5" not in _compiled:
        _compiled["v5"] = _build_nc()
    return _compiled["v5"]


def _host_prep(x, edge_weight, W0, W1, Wlin, edge_index):
    bf = ml_dtypes.bfloat16
    src = edge_index[0].astype(np.int64)
    tgt = edge_index[1].astype(np.int64)
    b = src // K
    sl = src - b * K
    tl = tgt - (tgt // K) * K

    # dense raw adjacency per graph, indexed [b, t, s]
    idx = (b * K + tl) * K + sl
    Araw = np.bincount(idx, weights=edge_weight.astype(np.float64),
                       minlength=B * K * K).astype(np.float32).reshape(B, K, K)
    deg = Araw.sum(axis=2)                      # weighted in-degree [B, K]
    with np.errstate(divide="ignore"):
        dinv = np.where(deg > 0, 1.0 / np.sqrt(deg), 0.0).astype(np.float32)
    An = Araw * dinv[:, :, None] * dinv[:, None, :]   # [b, t, s]
    ATn = np.ascontiguousarray(An.transpose(0, 2, 1))  # [b, s, t]

    # scatter graphs into per-core padded tile slots
    SLOTS = NT * G
    ATs = np.zeros((NCORES, SLOTS, K, K), np.float32)
    ATs[:, :GPC] = ATn.reshape(NCORES, GPC, K, K)
    ATs = ATs.reshape(NCORES, NT, G, K, K)

    at = np.zeros((NCORES, NT, PP, AW), np.float32)
    bd = at[:, :, :P, :P].reshape(NCORES, NT, G, K, G, K)
    atc = np.zeros((NCORES, NT, PP, CP), np.float32)
    cent = atc[:, :, :P, :G].reshape(NCORES, NT, G, K, G)
    for g in range(G):
        bd[:, :, g, :, g, :] = ATs[:, :, g]          # block-diagonal AT
        cent[:, :, g, :, g] = ATs[:, :, g, :, 0]     # center (t_local=0) col
    # device layout [PP, NT, .]
    at = np.ascontiguousarray(at.transpose(0, 2, 1, 3).astype(bf))
    atc = np.ascontiguousarray(atc.transpose(0, 2, 1, 3).astype(bf))

    # node-major x, tiled and padded: x_nm[p, i, f] = x[i*P + p, f], p < 125
    xp = np.zeros((NCORES, NT, PP, F0), np.float32)
    xtmp = np.zeros((NCORES, NT * P, F0), np.float32)
    xtmp[:, :GPC * K] = x.reshape(NCORES, GPC * K, F0)
    xp[:, :, :P, :] = xtmp.reshape(NCORES, NT, P, F0)
    x_nm = np.ascontiguousarray(xp.transpose(0, 2, 1, 3).astype(bf))

    # packed [w1 fic0 | w1 fic1 | wl]: wpk[p, fic*256+fo] = W1[fic*128+p, fo]
    wpk = np.empty((128, 2 * F1 + 2), np.float32)
    w1p = W1.reshape(2, 128, F1).transpose(1, 0, 2)   # [128, fic, fo]
    wpk[:, 0:F1] = w1p[:, 0, :]
    wpk[:, F1:2 * F1] = w1p[:, 1, :]
    wpk[:, 2 * F1:] = Wlin.reshape(2, 128).T          # [128, foc]
    wpk = np.ascontiguousarray(wpk.astype(bf))

    in_maps = []
    for c in range(NCORES):
        in_maps.append({
            "x": x_nm[c],
            "at": np.ascontiguousarray(at[c]),
            "atc": np.ascontiguousarray(atc[c]),
            "w0": np.ascontiguousarray(W0.astype(bf)),
            "wpk": wpk,
        })
    return in_maps


def _run(inputs, mode=None, trace=False):
    nc = _get_nc()
    in_maps = _host_prep(**inputs)
    res = run_bass_kernel_spmd(nc, in_maps, core_ids=list(range(NCORES)),
                               trace=trace)
    out = np.empty((B, 1), np.float32)
    for c in range(NCORES):
        vals = res.results[c]["out"].reshape(-1)
        out[c * GPC:(c + 1) * GPC, 0] = vals[:GPC]
    return out, res


def kernel(**inputs):
    out, _ = _run(inputs, trace=False)
    return out
